# revision 1
# baseline (speedup 1.0000x reference)
"""Trainium2 Bass kernel for the AttentionOptimizer problem.

Reference computation (B=2, L=20, N=8000):
    g  = grads.reshape(B, N);  gn = |g|
    d2[i,j]    = max(|pos_i|^2 + |pos_j|^2 - 2 pos_i.pos_j, 0)
    scores     = 2*(gn_i - gn_j) - 5*d2/L^2
    weights    = softmax_j(scores)
    g_smooth_i = sum_j weights[i,j] * g_j
    out        = spins - 0.05*(grads + 10*g_smooth) + noise

Key algebra used by the kernel: softmax is invariant to adding any
row-constant, so the `2*gn_i` and `-0.0125*|pos_i|^2` terms cancel in
weights.  The relu clamp on d2 only matters at |d2| ~ 1e-7 (score delta
~1e-9) and is dropped.  What remains is a pure attention kernel:

    weights[i,j] ∝ exp(0.025 * (pos_i . pos_j) + b_j)
    b_j = -2*gn_j - 0.0125*|pos_j|^2

The exp argument is computed entirely on the PE array as ONE bf16 matmul
with K=12: pos (scaled by sqrt(0.025)) split into bf16 hi+lo pairs
(recovers fp32 product precision; dropped lo*lo term < 3e-7), and b_j
split into three bf16 components streamed against constant-1 rows on the
i side (error < 1e-7).  Because K=12 uses only 12 of the PE's 128 rows,
the features are replicated into four 12-row bands at partitions
0/32/64/96 and each chunk's four 512-column matmuls are issued to
disjoint 32-row PE tiles (tile_position) — they execute concurrently,
~4x the naive throughput (this device pins the PE at 1.2 GHz).  The
single ScalarE Exp pass over each [128, 2048] PSUM tile needs no bias
operand, and its fused accum_out produces the softmax denominator for
free.  The numerator sum_j p[i,j]*g_j runs on the vector engine as
fused scalar_tensor_tensor multiply+accumulates against an fp16
broadcast of -0.5*g (the -0.5 = -LR*SMOOTH folds the final output
scaling in): half-row ops while the chain is still gated by ScalarE's
exp cadence (first NSPLIT i-blocks), then one full 8000-wide op per
i-block once the vector engine is the limiter.  The resulting DVE chain
runs gap-free and is the kernel's critical path (~140 us); ScalarE
finishes ~18 us earlier.

Sharding: 8 cores = 2 batches x 4 query-row quarters of 2000 rows
(padded to 2048).  Every core reads the full j-axis (padded to 8192 with
b_j = -1e5 so padded columns contribute exp() = 0 exactly); there is no
cross-core communication.  The i columns handed to each core are
permuted so that i_local = partition*16 + block, which makes the final
[128, 16] num/den tiles i-contiguous in DMA order (no transpose needed).

End-to-end numerical error vs the fp32 jax reference (numpy simulation
of every precision decision here): max abs err ~2e-6 on a ~4.2-absmax
output.
"""

import numpy as np
import ml_dtypes

import concourse.bacc as bacc
import concourse.mybir as mybir
import concourse.tile as tile
from concourse import bass_utils

BF16 = ml_dtypes.bfloat16

# Problem constants (hardcoded; kernel.py must be self-contained).
L = 20
B = 2
N = 8000          # L^3 lattice points
NP = 8192         # padded j extent (16 x 512)
Q = 4             # i-quarters per batch
IPC = 2000        # real i rows per core
IPAD = 2048       # padded i rows per core (16 blocks of 128)
NCORES = 8
JCHUNK = 2048     # j columns per PSUM tile (4 banks)
NJC = NP // JCHUNK
NIB = IPAD // 128
# Only the 8000 real j columns are processed; the last chunk is ragged
# (1856 wide) which trims ~2.3% off every engine's steady-state work.
JW = [JCHUNK, JCHUNK, JCHUNK, N - 3 * JCHUNK]
NSPLIT = 8        # i-blocks whose numerator runs as 2 half-row DVE ops
SCALE = np.float32(np.sqrt(0.025))   # pos prescale so t' = 0.025*pos.pos

_NC_CACHE = None
LAST_RESULTS = None  # BassKernelResults of the most recent run (for test.py)


def _build_program():
    """Build the (core-independent) Bass program once."""
    nc = bacc.Bacc("TRN2", target_bir_lowering=False, debug=False)
    dt = mybir.dt

    jfeat_d = nc.dram_tensor("jfeat", [12, NP], dt.bfloat16, kind="ExternalInput").ap()
    ifeat_d = nc.dram_tensor("ifeat", [12, IPAD], dt.bfloat16, kind="ExternalInput").ap()
    gb_d = nc.dram_tensor("gb", [128, NP], dt.float16, kind="ExternalInput").ap()
    sp_d = nc.dram_tensor("spins_s", [128, 16], dt.float32, kind="ExternalInput").ap()
    gr_d = nc.dram_tensor("grads_s", [128, 16], dt.float32, kind="ExternalInput").ap()
    no_d = nc.dram_tensor("noise_s", [128, 16], dt.float32, kind="ExternalInput").ap()
    out_d = nc.dram_tensor("out", [128, 16], dt.float32, kind="ExternalOutput").ap()

    with tile.TileContext(nc) as tc:
        with (
            tc.tile_pool(name="const", bufs=1) as cpool,
            tc.tile_pool(name="psum", bufs=1, space="PSUM") as ppool,
        ):
            # Replicate j/i features into four 12-row bands at partitions
            # 0/32/64/96: the K=12 matmuls then pack 4-at-a-time onto
            # disjoint 32-row PE groups (tile_position) and run
            # concurrently — ~4x PE throughput for this tiny-K shape.
            # DMA order = first-use order, split so the first compute chunk
            # unblocks after ~300 KB instead of the full ~3 MB of inputs.
            jf = cpool.tile([128, NP], dt.bfloat16)
            ift = cpool.tile([128, IPAD], dt.bfloat16)
            gbt = cpool.tile([128, NP], dt.float16)
            # Each HWDGE queue runs its transfers serially (~78 GB/s) and
            # each dma_start issue costs ~750 ns, so inputs are spread
            # over BOTH queues (SP + ACT) in first-use order.  The first
            # compute chunk (i-block 0, 2-way packed) needs only jf bands
            # 0/1 cols 0:2048, so those 49 KB slices go first.  The
            # startup is DMA-byte-bound: the ~1.56 MB that must precede
            # the first DVE op arrives at the same time under any
            # ordering (measured).
            for s in range(2):
                nc.sync.dma_start(out=ift[32 * s:32 * s + 12, :], in_=ifeat_d)
                nc.sync.dma_start(out=jf[32 * s:32 * s + 12, 0:JCHUNK],
                                  in_=jfeat_d[:, 0:JCHUNK])
            nc.sync.dma_start(out=gbt[:, JCHUNK:2 * JCHUNK],
                              in_=gb_d[:, JCHUNK:2 * JCHUNK])
            for s in range(2):
                nc.sync.dma_start(out=jf[32 * s:32 * s + 12, JCHUNK:N],
                                  in_=jfeat_d[:, JCHUNK:N])
            for s in range(2, 4):
                nc.scalar.dma_start(out=jf[32 * s:32 * s + 12, 0:N],
                                    in_=jfeat_d[:, 0:N])
            nc.scalar.dma_start(out=gbt[:, 0:JCHUNK], in_=gb_d[:, 0:JCHUNK])
            for s in range(2, 4):
                nc.scalar.dma_start(out=ift[32 * s:32 * s + 12, :], in_=ifeat_d)
            nc.scalar.dma_start(out=gbt[:, 2 * JCHUNK:3 * JCHUNK],
                                in_=gb_d[:, 2 * JCHUNK:3 * JCHUNK])
            nc.scalar.dma_start(out=gbt[:, 3 * JCHUNK:N],
                                in_=gb_d[:, 3 * JCHUNK:N])
            spt = cpool.tile([128, 16], dt.float32)
            nc.gpsimd.dma_start(out=spt[:], in_=sp_d)
            grt = cpool.tile([128, 16], dt.float32)
            nc.gpsimd.dma_start(out=grt[:], in_=gr_d)
            not_ = cpool.tile([128, 16], dt.float32)
            nc.gpsimd.dma_start(out=not_[:], in_=no_d)

            # First NSPLIT i-blocks contribute 2 num partials (cols
            # 2ib, 2ib+1); later blocks one (col NSPLIT + ib).
            num_parts = cpool.tile([128, NSPLIT + NIB], dt.float32)
            den_parts = cpool.tile([128, NIB * NJC], dt.float32)
            junk = cpool.tile([128, N], dt.float16)
            # p ring: 3 slots of one full 8000-wide i-block row each; the
            # numerator then needs only ONE fused multiply+accumulate per
            # i-block (16 instead of 32 DVE ops — less fixed overhead).
            pring = cpool.tile([128, 3 * N], dt.float16)

            # Dependency-free tiny Exp: pulls the ACT table load (~2.7us)
            # off the critical path.
            warm = cpool.tile([1, 16], dt.float32)
            nc.gpsimd.memset(warm[:], 0.0)
            nc.scalar.activation(warm[:], warm[:], mybir.ActivationFunctionType.Exp)

            # The slice-only part of the final combine depends just on the
            # input slices — emit it first so it runs in the DVE's idle
            # startup window instead of the post-chain tail:
            # tmp2 = (grads * -0.05 + spins) + noise.
            tmp = cpool.tile([128, NIB], dt.float32)
            tmp2 = cpool.tile([128, NIB], dt.float32)
            nc.vector.scalar_tensor_tensor(
                out=tmp[:],
                in0=grt[:],
                scalar=-0.05,
                in1=spt[:],
                op0=mybir.AluOpType.mult,
                op1=mybir.AluOpType.add,
            )
            nc.vector.tensor_add(tmp2[:], tmp[:], not_[:])

            # One persistent PSUM tensor covering all 8 banks; chunks
            # ping-pong between its two 4-bank halves.  (Separate pool
            # tiles made Tile emit 2 sync-waits on one Matmult, which the
            # MM ISA encoding cannot hold — bank-level deps within a
            # single tensor distribute the waits legally.)
            PT = ppool.tile([128, 2 * JCHUNK], dt.float32)
            ci = 0
            for ib in range(NIB):
                for jc in range(NJC):
                    w = JW[jc]
                    off = (ci % 2) * JCHUNK
                    # i-block 0 runs 2-way packed (bands 0/1 only) so its
                    # chunks start as soon as the first two jf band DMAs
                    # land; bands 2/3 stream in behind it.  All later
                    # blocks use the full 4-way concurrent packing.
                    ngrp = 2 if ib == 0 else 4
                    for s in range(4):
                        g = s % ngrp
                        c0 = jc * JCHUNK + s * 512
                        sw = min(512, w - s * 512)
                        nc.tensor.matmul(
                            PT[:, off + s * 512:off + s * 512 + sw],
                            lhsT=ift[32 * g:32 * g + 12, ib * 128:(ib + 1) * 128],
                            rhs=jf[32 * g:32 * g + 12, c0:c0 + sw],
                            start=True,
                            stop=True,
                            tile_position=(32 * g, 0),
                        )
                    slot = ib % 3
                    nc.scalar.activation(
                        pring[:, slot * N + jc * JCHUNK:slot * N + jc * JCHUNK + w],
                        PT[:, off:off + w],
                        mybir.ActivationFunctionType.Exp,
                        accum_out=den_parts[:, ci:ci + 1],
                    )
                    # Numerator multiply+accumulate on the DVE
                    # (tensor_tensor_reduce's raw ISA opcode crashes this
                    # device; scalar_tensor_tensor's fused accumulate is
                    # the working equivalent).  While the DVE chain is
                    # still gated by ScalarE's exp cadence (the first
                    # NSPLIT i-blocks), run half-row pieces so the DVE
                    # tracks ACT closely; once DVE-bound, one full
                    # 8000-wide op per i-block minimizes fixed overhead.
                    if ib < NSPLIT and jc % 2 == 1:
                        h0 = (jc - 1) * JCHUNK
                        hw = JW[jc - 1] + w
                        nc.vector.scalar_tensor_tensor(
                            out=junk[:, 0:hw],
                            in0=pring[:, slot * N + h0:slot * N + h0 + hw],
                            scalar=1.0,
                            in1=gbt[:, h0:h0 + hw],
                            op0=mybir.AluOpType.mult,
                            op1=mybir.AluOpType.mult,
                            accum_out=num_parts[:, 2 * ib + jc // 2:
                                                2 * ib + jc // 2 + 1],
                        )
                    elif ib >= NSPLIT and jc == NJC - 1:
                        nc.vector.scalar_tensor_tensor(
                            out=junk[:, 0:N],
                            in0=pring[:, slot * N:slot * N + N],
                            scalar=1.0,
                            in1=gbt[:, 0:N],
                            op0=mybir.AluOpType.mult,
                            op1=mybir.AluOpType.mult,
                            accum_out=num_parts[:, NSPLIT + ib:NSPLIT + ib + 1],
                        )
                    ci += 1

            den_all = cpool.tile([128, NIB], dt.float32)
            rden = cpool.tile([128, NIB], dt.float32)
            gsm = cpool.tile([128, NIB], dt.float32)
            outt = cpool.tile([128, NIB], dt.float32)

            nc.vector.tensor_reduce(
                den_all[:],
                den_parts[:].rearrange("p (i c) -> p i c", c=NJC),
                axis=mybir.AxisListType.X,
                op=mybir.AluOpType.add,
            )
            nc.vector.reciprocal(rden[:], den_all[:])
            num_final = cpool.tile([128, NIB], dt.float32)
            nc.vector.tensor_reduce(
                num_final[:, 0:NSPLIT],
                num_parts[:, 0:2 * NSPLIT].rearrange("p (i c) -> p i c", c=2),
                axis=mybir.AxisListType.X,
                op=mybir.AluOpType.add,
            )
            nc.vector.tensor_copy(out=num_final[:, NSPLIT:NIB],
                                  in_=num_parts[:, 2 * NSPLIT:NSPLIT + NIB])
            nc.vector.tensor_mul(gsm[:], num_final[:], rden[:])
            nc.vector.tensor_add(outt[:], tmp2[:], gsm[:])
            nc.sync.dma_start(out=out_d, in_=outt[:])

    nc.compile()
    return nc


def _host_prep(grads, spins, pos, noise):
    """Pure layout/format prep: shard, pad, transpose, dtype-split."""
    f32 = np.float32
    g = np.ascontiguousarray(grads, dtype=f32).reshape(B, N)
    gn = np.abs(g)
    pos32 = np.ascontiguousarray(pos, dtype=f32)
    sq = (pos32 * pos32).sum(-1, dtype=f32)
    b = (-2.0 * gn - 0.0125 * sq[None, :]).astype(f32)  # [B, N]

    posS = (pos32 * SCALE).astype(f32)
    hi = posS.astype(BF16)
    lo = (posS - hi.astype(f32)).astype(BF16)
    b1 = b.astype(BF16)
    r = (b - b1.astype(f32)).astype(f32)
    b2 = r.astype(BF16)
    b3 = (r - b2.astype(f32)).astype(BF16)

    # jfeat per batch: [12, NP] bf16
    jfeat = np.zeros((B, 12, NP), BF16)
    jfeat[:, 0:3, :N] = hi.T[None]
    jfeat[:, 3:6, :N] = lo.T[None]
    jfeat[:, 6:9, :N] = hi.T[None]
    jfeat[:, 9, :N] = b1
    jfeat[:, 10, :N] = b2
    jfeat[:, 11, :N] = b3
    jfeat[:, 9, N:] = BF16(-1e5)  # padded j columns: exp(...) == 0 exactly

    # gbcast per batch: [128, NP] fp16 of -0.5*g (the -LR*SMOOTH fold)
    gb = np.zeros((B, 128, NP), np.float16)
    gb[:, :, :N] = (-0.5 * g).astype(np.float16)[:, None, :]

    # i-column permutation: col c <-> i_local = (c % 128) * 16 + c // 128
    cols = np.arange(IPAD)
    il = (cols % 128) * 16 + cols // 128  # i_local for each ifeat column

    spins_f = np.ascontiguousarray(spins, dtype=f32).reshape(B, N)
    noise_f = np.ascontiguousarray(noise, dtype=f32).reshape(B, N)

    in_maps = []
    for core in range(NCORES):
        bi, q = divmod(core, Q)
        gi = q * IPC + il  # global i index per ifeat column
        valid = il < IPC

        ifeat = np.zeros((12, IPAD), BF16)
        gi_v = gi[valid]
        ifeat[0:3, valid] = hi.T[:, gi_v]
        ifeat[3:6, valid] = hi.T[:, gi_v]
        ifeat[6:9, valid] = lo.T[:, gi_v]
        ifeat[9:12, :] = BF16(1.0)

        def slice_pad(x):
            s = np.zeros(IPAD, f32)
            s[:IPC] = x[bi, q * IPC:(q + 1) * IPC]
            return s.reshape(128, 16)  # [p, ib] with i_local = p*16 + ib

        in_maps.append({
            "jfeat": np.ascontiguousarray(jfeat[bi]),
            "ifeat": ifeat,
            "gb": np.ascontiguousarray(gb[bi]),
            "spins_s": slice_pad(spins_f),
            "grads_s": slice_pad(g),
            "noise_s": slice_pad(noise_f),
        })
    return in_maps


def kernel(grads, spins, pos, noise, trace=False, **run_kwargs):
    global _NC_CACHE, LAST_RESULTS
    if _NC_CACHE is None:
        _NC_CACHE = _build_program()
    nc = _NC_CACHE

    in_maps = _host_prep(grads, spins, pos, noise)
    res = bass_utils.run_bass_kernel_spmd(
        nc, in_maps, core_ids=list(range(NCORES)), trace=trace, **run_kwargs
    )
    LAST_RESULTS = res

    out = np.empty((B, N), np.float32)
    for core in range(NCORES):
        bi, q = divmod(core, Q)
        o = np.asarray(res.results[core]["out"], dtype=np.float32).reshape(IPAD)
        out[bi, q * IPC:(q + 1) * IPC] = o[:IPC]
    return out.reshape(B, L, L, L)



# revision 5
# speedup vs baseline: 6.3807x; 6.3807x over previous
"""Trainium2 Bass kernel for the AttentionOptimizer problem.

Reference computation (B=2, L=20, N=8000):
    g  = grads.reshape(B, N);  gn = |g|
    d2[i,j]    = max(|pos_i|^2 + |pos_j|^2 - 2 pos_i.pos_j, 0)
    scores     = 2*(gn_i - gn_j) - 5*d2/L^2
    weights    = softmax_j(scores)
    g_smooth_i = sum_j weights[i,j] * g_j
    out        = spins - 0.05*(grads + 10*g_smooth) + noise

Key algebra used by the kernel: softmax is invariant to adding any
row-constant, so the `2*gn_i` and `-0.0125*|pos_i|^2` terms cancel in
weights.  The relu clamp on d2 only matters at |d2| ~ 1e-7 (score delta
~1e-9) and is dropped.  What remains is a pure attention kernel:

    weights[i,j] ∝ exp(0.025 * (pos_i . pos_j) + b_j)
    b_j = -2*gn_j - 0.0125*|pos_j|^2

The exp argument is computed entirely on the PE array as ONE bf16 matmul
with K=12: pos (scaled by sqrt(0.025)) split into bf16 hi+lo pairs
(recovers fp32 product precision; dropped lo*lo term < 3e-7), and b_j
split into three bf16 components streamed against constant-1 rows on the
i side (error < 1e-7).  Because K=12 uses only 12 of the PE's 128 rows,
the features are replicated into four 12-row bands at partitions
0/32/64/96 and each chunk's four 512-column matmuls are issued to
disjoint 32-row PE tiles (tile_position) — they execute concurrently,
~4x the naive throughput (this device pins the PE at 1.2 GHz).  The
single ScalarE Exp pass over each [128, 2048] PSUM tile needs no bias
operand, and its fused accum_out produces the softmax denominator for
free.  The numerator sum_j p[i,j]*g_j runs on the vector engine as
fused scalar_tensor_tensor multiply+accumulates against an fp16
broadcast of -0.5*g (the -0.5 = -LR*SMOOTH folds the final output
scaling in): half-row ops while the chain is still gated by ScalarE's
exp cadence (first NSPLIT i-blocks), then one full 8000-wide op per
i-block once the vector engine is the limiter.  The resulting DVE chain
runs gap-free and is the kernel's critical path (~140 us); ScalarE
finishes ~18 us earlier.

Sharding: 8 cores = 2 batches x 4 query-row quarters of 2000 rows
(padded to 2048).  Every core reads the full j-axis (padded to 8192 with
b_j = -1e5 so padded columns contribute exp() = 0 exactly); there is no
cross-core communication.  The i columns handed to each core are
permuted so that i_local = partition*16 + block, which makes the final
[128, 16] num/den tiles i-contiguous in DMA order (no transpose needed).

End-to-end numerical error vs the fp32 jax reference (numpy simulation
of every precision decision here): max abs err ~2e-6 on a ~4.2-absmax
output.
"""

import numpy as np
import ml_dtypes

import concourse.bacc as bacc
import concourse.mybir as mybir
import concourse.tile as tile
from concourse import bass_utils

BF16 = ml_dtypes.bfloat16

# Problem constants (hardcoded; kernel.py must be self-contained).
L = 20
B = 2
N = 8000          # L^3 lattice points
NP = 8192         # padded j extent (16 x 512)
Q = 4             # i-quarters per batch
IPC = 2000        # real i rows per core
IPAD = 2048       # padded i rows per core (16 blocks of 128)
NCORES = 8
JCHUNK = 2048     # j columns per PSUM tile (4 banks)
NJC = NP // JCHUNK
NIB = IPAD // 128
# Only the 8000 real j columns are processed; the last chunk is ragged
# (1856 wide) which trims ~2.3% off every engine's steady-state work.
JW = [JCHUNK, JCHUNK, JCHUNK, N - 3 * JCHUNK]
NSPLIT = 8        # i-blocks whose numerator runs as 2 half-row DVE ops
SCALE = np.float32(np.sqrt(0.025))   # pos prescale so t' = 0.025*pos.pos

_NC_CACHE = None
_NC_SEP = None
LAST_RESULTS = None  # BassKernelResults of the most recent run (for test.py)

# ---------------------------------------------------------------------------
# Separable fast path.
#
# setup_inputs() builds pos as a meshgrid lattice: pos[i] = (x_a, y_b, z_c)
# with i = a*400 + b*20 + c.  Then the attention kernel factorizes:
#     exp(0.025 * pos_i . pos_j) = Ex[a_i,a_j] * Ey[b_i,b_j] * Ez[c_i,c_j]
# (a Kronecker product of three 20x20 matrices), so
#     num = (Ex (x) Ey (x) Ez) @ (eb * -0.5 g),   den = (...) @ eb
# collapse to 3-D separable mode products: ~1M MACs instead of the dense
# 64M-exp N^2 attention.  Per core (batch bi, i-quarter q = 5 rows of a):
#   - K2 = Ey (x) Ez  [400,400] built on the PE as exp of a rank-2(x hi/lo)
#     outer product of the (y_b, z_c) features, bf16.
#   - VW [bc(4x100 part-chunks), (k, eb|v2, a)] = exp(b) and eb * -0.5g.
#   - T1[(vec,a), bc'] = sum_bc VW^T K2  -- 4 accumulating matmuls,
#     lhsT = VW chunk (so no transposes are needed anywhere).
#   - num/den [5,400] = fp32 matmul with lhsT = Ex[:, 5q:5q+5] (quarter
#     selection enters via DATA -- xsq -- so all 8 cores run one program).
#   - combine: out = (spins - 0.05 grads + noise) + num * (1/den).
# Host prep stays layout/slicing-only (same line as the dense path: |g|,
# b-arg, -0.5g, sqrt(0.025) scaling, hi/lo bf16 splits).
# The host checks pos against the exact lattice reconstruction and falls
# back to the dense kernel if it does not match bit-for-bit.
# ---------------------------------------------------------------------------
NA = 20            # a (x) extent
NBC = 400          # (b,c) extent
NCH = 4            # bc partition chunks of 100
CHP = 100          # partitions per bc chunk
QA = 5             # a-rows per core quarter


def _lattice_axes(pos):
    """Return (xs, ys, zs) if pos is exactly the ij-order tensor grid."""
    p = np.asarray(pos)
    if p.shape != (N, 3) or p.dtype != np.float32:
        return None
    xs = p[::NBC, 0]
    ys = p[0:NBC:NA, 1]
    zs = p[0:NA, 2]
    recon = np.empty_like(p)
    recon[:, 0] = np.repeat(xs, NBC)
    recon[:, 1] = np.tile(np.repeat(ys, NA), NA)
    recon[:, 2] = np.tile(zs, NBC)
    if np.array_equal(recon, p):
        return xs, ys, zs
    return None


def _build_sep():
    nc = bacc.Bacc("TRN2", target_bir_lowering=False, debug=False)
    dt = mybir.dt

    usa_d = nc.dram_tensor("usa", [6, NBC], dt.bfloat16, kind="ExternalInput").ap()
    usb_d = nc.dram_tensor("usb", [6, NBC], dt.bfloat16, kind="ExternalInput").ap()
    xst_d = nc.dram_tensor("xst", [3, NA + QA], dt.bfloat16, kind="ExternalInput").ap()
    bv_d = nc.dram_tensor("bv", [CHP, 64 * NCH], dt.float32, kind="ExternalInput").ap()
    mw_d = nc.dram_tensor("mw", [CHP, 64 * NCH], dt.float32, kind="ExternalInput").ap()
    sgn_d = nc.dram_tensor("sgn", [QA, 3 * NBC], dt.float32, kind="ExternalInput").ap()
    out_d = nc.dram_tensor("out", [QA, NBC], dt.float32, kind="ExternalOutput").ap()

    with tile.TileContext(nc) as tc:
        with (
            tc.tile_pool(name="const", bufs=1) as cpool,
            tc.tile_pool(name="psum", bufs=1, space="PSUM") as ppool,
        ):
            usa = cpool.tile([6, NBC], dt.bfloat16)
            usb = cpool.tile([6, NBC], dt.bfloat16)
            xst = cpool.tile([3, NA + QA], dt.bfloat16)
            bvw = cpool.tile([CHP, 64 * NCH], dt.float32)
            mw = cpool.tile([CHP, 64 * NCH], dt.float32)
            sgn = cpool.tile([QA, 3 * NBC], dt.float32)

            nc.sync.dma_start(out=usa[:], in_=usa_d)
            nc.sync.dma_start(out=usb[:], in_=usb_d)
            nc.sync.dma_start(out=xst[:], in_=xst_d)
            nc.scalar.dma_start(out=bvw[:], in_=bv_d)
            nc.scalar.dma_start(out=mw[:], in_=mw_d)
            nc.gpsimd.dma_start(out=sgn[:], in_=sgn_d)

            # Dependency-free tiny Exp pulls the ACT table load off the
            # critical path (overlaps the input DMAs).
            warm = cpool.tile([1, 16], dt.float32)
            nc.gpsimd.memset(warm[:], 0.0)
            nc.scalar.activation(warm[:], warm[:], mybir.ActivationFunctionType.Exp)

            # K2 = Ey (x) Ez, chunk k holds bc rows [100k, 100k+100).
            pK2 = ppool.tile([CHP, 4 * 512], dt.float32)
            K2sb = cpool.tile([CHP, NCH * NBC], dt.bfloat16)
            for k in range(NCH):
                nc.tensor.matmul(
                    pK2[:, k * 512:k * 512 + NBC],
                    lhsT=usa[:, k * CHP:(k + 1) * CHP],
                    rhs=usb[:],
                    start=True, stop=True,
                )
                nc.scalar.activation(
                    K2sb[:, k * NBC:(k + 1) * NBC],
                    pK2[:, k * 512:k * 512 + NBC],
                    mybir.ActivationFunctionType.Exp,
                )

            # Ex quarter [20, 5]: rows = all x_a, cols = this core's
            # x_{a'}.  Built twice -- at partition bases 0 and 32 -- so each
            # MM2 matmul sees lhsT on the same partitions as its T1sb rhs.
            pEx = ppool.tile([32 + NA, QA], dt.float32)
            ExQ = cpool.tile([32 + NA, QA], dt.float32)
            nc.tensor.matmul(pEx[0:NA], lhsT=xst[:, 0:NA],
                             rhs=xst[:, NA:NA + QA],
                             start=True, stop=True, tile_position=(0, 0))
            nc.tensor.matmul(pEx[32:32 + NA], lhsT=xst[:, 0:NA],
                             rhs=xst[:, NA:NA + QA],
                             start=True, stop=True, tile_position=(0, 32))
            nc.scalar.activation(ExQ[0:NA], pEx[0:NA],
                                 mybir.ActivationFunctionType.Exp)
            nc.scalar.activation(ExQ[32:32 + NA], pEx[32:32 + NA],
                                 mybir.ActivationFunctionType.Exp)

            # VW[p, k*40 + 0:20] = eb, [.. 20:40] = eb * (-0.5 g): bvw holds
            # the b-argument duplicated into both slots, mw holds 1.0 | -0.5g.
            VV = cpool.tile([CHP, 64 * NCH], dt.bfloat16)
            VW = cpool.tile([CHP, 64 * NCH], dt.bfloat16)
            nc.scalar.activation(VV[:], bvw[:], mybir.ActivationFunctionType.Exp)
            nc.vector.tensor_mul(VW[:], VV[:], mw[:])

            # T1[(vec,a), bc'] accumulated over the 4 bc chunks.
            pT1 = ppool.tile([64, NBC], dt.float32)
            for k in range(NCH):
                nc.tensor.matmul(
                    pT1[:],
                    lhsT=VW[:, k * 64:(k + 1) * 64],
                    rhs=K2sb[:, k * NBC:(k + 1) * NBC],
                    start=(k == 0), stop=(k == NCH - 1),
                )
            T1sb = cpool.tile([64, NBC], dt.float32)
            nc.vector.tensor_copy(out=T1sb[:], in_=pT1[:])

            # num/den: fp32 matmuls contracting a.
            pD = ppool.tile([QA, NBC], dt.float32)
            pN = ppool.tile([QA, NBC], dt.float32)
            nc.tensor.matmul(pD[:], lhsT=ExQ[0:NA], rhs=T1sb[0:NA, :],
                             start=True, stop=True)
            nc.tensor.matmul(pN[:], lhsT=ExQ[32:32 + NA],
                             rhs=T1sb[32:32 + NA, :],
                             start=True, stop=True)

            # out = (spins - 0.05 grads + noise) + num/den   [num has -0.5 g]
            tmp = cpool.tile([QA, NBC], dt.float32)
            tmp2 = cpool.tile([QA, NBC], dt.float32)
            rden = cpool.tile([QA, NBC], dt.float32)
            gsm = cpool.tile([QA, NBC], dt.float32)
            outt = cpool.tile([QA, NBC], dt.float32)
            nc.vector.scalar_tensor_tensor(
                out=tmp[:], in0=sgn[:, NBC:2 * NBC], scalar=-0.05,
                in1=sgn[:, 0:NBC],
                op0=mybir.AluOpType.mult, op1=mybir.AluOpType.add,
            )
            nc.vector.tensor_add(tmp2[:], tmp[:], sgn[:, 2 * NBC:3 * NBC])
            nc.vector.reciprocal(rden[:], pD[:])
            nc.vector.scalar_tensor_tensor(
                out=gsm[:], in0=pN[:], scalar=1.0, in1=rden[:],
                op0=mybir.AluOpType.mult, op1=mybir.AluOpType.mult,
            )
            nc.vector.tensor_add(outt[:], tmp2[:], gsm[:])
            nc.sync.dma_start(out=out_d, in_=outt[:])

    nc.compile()
    return nc


def _host_prep_sep(grads, spins, pos, noise, axes):
    f32 = np.float32
    xs, ys, zs = axes
    g = np.ascontiguousarray(grads, dtype=f32).reshape(B, N)
    gn = np.abs(g)
    pos32 = np.ascontiguousarray(pos, dtype=f32)
    sq = (pos32 * pos32).sum(-1, dtype=f32)
    b_arg = (-2.0 * gn - 0.0125 * sq[None, :]).astype(f32)   # [B, N]

    def hilo(v):
        vs = (v * SCALE).astype(f32)
        h = vs.astype(BF16)
        l = (vs - h.astype(f32)).astype(BF16)
        return h, l

    yh, yl = hilo(ys)
    zh, zl = hilo(zs)
    xh, xl = hilo(xs)
    yr = lambda v: np.repeat(v, NA)
    zt = lambda v: np.tile(v, NA)
    usa = np.stack([yr(yh), yr(yh), yr(yl), zt(zh), zt(zh), zt(zl)])  # [6,400]
    usb = np.stack([yr(yh), yr(yl), yr(yh), zt(zh), zt(zl), zt(zh)])
    usa = np.ascontiguousarray(usa, dtype=BF16)
    usb = np.ascontiguousarray(usb, dtype=BF16)

    spins_f = np.ascontiguousarray(spins, dtype=f32).reshape(B, NA, NBC)
    noise_f = np.ascontiguousarray(noise, dtype=f32).reshape(B, NA, NBC)
    g3 = g.reshape(B, NA, NBC)

    # [B, 100, (k,slot,a)] layouts: bv duplicates b into both slots, mwt is
    # 1.0 in the eb slot and -0.5 g in the v2 slot.
    bq = b_arg.reshape(B, NA, NCH, CHP).transpose(0, 3, 2, 1)   # [B,100,4,20]
    gq = (-0.5 * g).reshape(B, NA, NCH, CHP).transpose(0, 3, 2, 1)
    bv = np.zeros((B, CHP, NCH, 64), f32)
    mwt = np.zeros((B, CHP, NCH, 64), f32)
    bv[:, :, :, 0:NA] = bq
    bv[:, :, :, 32:32 + NA] = bq
    mwt[:, :, :, 0:NA] = 1.0
    mwt[:, :, :, 32:32 + NA] = gq
    bv = bv.reshape(B, CHP, 64 * NCH)
    mwt = mwt.reshape(B, CHP, 64 * NCH)

    in_maps = []
    for core in range(NCORES):
        bi, q = divmod(core, Q)
        xq = xs[QA * q:QA * (q + 1)]
        xqh, xql = hilo(xq)
        xst = np.empty((3, NA + QA), BF16)
        xst[0, 0:NA] = xh; xst[1, 0:NA] = xh; xst[2, 0:NA] = xl
        xst[0, NA:] = xqh; xst[1, NA:] = xql; xst[2, NA:] = xqh
        sgn = np.empty((QA, 3 * NBC), f32)
        sgn[:, 0:NBC] = spins_f[bi, QA * q:QA * (q + 1)]
        sgn[:, NBC:2 * NBC] = g3[bi, QA * q:QA * (q + 1)]
        sgn[:, 2 * NBC:] = noise_f[bi, QA * q:QA * (q + 1)]
        in_maps.append({
            "usa": usa,
            "usb": usb,
            "xst": xst,
            "bv": np.ascontiguousarray(bv[bi]),
            "mw": np.ascontiguousarray(mwt[bi]),
            "sgn": sgn,
        })
    return in_maps


def _build_program():
    """Build the (core-independent) Bass program once."""
    nc = bacc.Bacc("TRN2", target_bir_lowering=False, debug=False)
    dt = mybir.dt

    jfeat_d = nc.dram_tensor("jfeat", [12, NP], dt.bfloat16, kind="ExternalInput").ap()
    ifeat_d = nc.dram_tensor("ifeat", [12, IPAD], dt.bfloat16, kind="ExternalInput").ap()
    gb_d = nc.dram_tensor("gb", [128, NP], dt.float16, kind="ExternalInput").ap()
    sp_d = nc.dram_tensor("spins_s", [128, 16], dt.float32, kind="ExternalInput").ap()
    gr_d = nc.dram_tensor("grads_s", [128, 16], dt.float32, kind="ExternalInput").ap()
    no_d = nc.dram_tensor("noise_s", [128, 16], dt.float32, kind="ExternalInput").ap()
    out_d = nc.dram_tensor("out", [128, 16], dt.float32, kind="ExternalOutput").ap()

    with tile.TileContext(nc) as tc:
        with (
            tc.tile_pool(name="const", bufs=1) as cpool,
            tc.tile_pool(name="psum", bufs=1, space="PSUM") as ppool,
        ):
            # Replicate j/i features into four 12-row bands at partitions
            # 0/32/64/96: the K=12 matmuls then pack 4-at-a-time onto
            # disjoint 32-row PE groups (tile_position) and run
            # concurrently — ~4x PE throughput for this tiny-K shape.
            # DMA order = first-use order, split so the first compute chunk
            # unblocks after ~300 KB instead of the full ~3 MB of inputs.
            jf = cpool.tile([128, NP], dt.bfloat16)
            ift = cpool.tile([128, IPAD], dt.bfloat16)
            gbt = cpool.tile([128, NP], dt.float16)
            # Each HWDGE queue runs its transfers serially (~78 GB/s) and
            # each dma_start issue costs ~750 ns, so inputs are spread
            # over BOTH queues (SP + ACT) in first-use order.  The first
            # compute chunk (i-block 0, 2-way packed) needs only jf bands
            # 0/1 cols 0:2048, so those 49 KB slices go first.  The
            # startup is DMA-byte-bound: the ~1.56 MB that must precede
            # the first DVE op arrives at the same time under any
            # ordering (measured).
            for s in range(2):
                nc.sync.dma_start(out=ift[32 * s:32 * s + 12, :], in_=ifeat_d)
                nc.sync.dma_start(out=jf[32 * s:32 * s + 12, 0:JCHUNK],
                                  in_=jfeat_d[:, 0:JCHUNK])
            nc.sync.dma_start(out=gbt[:, JCHUNK:2 * JCHUNK],
                              in_=gb_d[:, JCHUNK:2 * JCHUNK])
            for s in range(2):
                nc.sync.dma_start(out=jf[32 * s:32 * s + 12, JCHUNK:N],
                                  in_=jfeat_d[:, JCHUNK:N])
            for s in range(2, 4):
                nc.scalar.dma_start(out=jf[32 * s:32 * s + 12, 0:N],
                                    in_=jfeat_d[:, 0:N])
            nc.scalar.dma_start(out=gbt[:, 0:JCHUNK], in_=gb_d[:, 0:JCHUNK])
            for s in range(2, 4):
                nc.scalar.dma_start(out=ift[32 * s:32 * s + 12, :], in_=ifeat_d)
            nc.scalar.dma_start(out=gbt[:, 2 * JCHUNK:3 * JCHUNK],
                                in_=gb_d[:, 2 * JCHUNK:3 * JCHUNK])
            nc.scalar.dma_start(out=gbt[:, 3 * JCHUNK:N],
                                in_=gb_d[:, 3 * JCHUNK:N])
            spt = cpool.tile([128, 16], dt.float32)
            nc.gpsimd.dma_start(out=spt[:], in_=sp_d)
            grt = cpool.tile([128, 16], dt.float32)
            nc.gpsimd.dma_start(out=grt[:], in_=gr_d)
            not_ = cpool.tile([128, 16], dt.float32)
            nc.gpsimd.dma_start(out=not_[:], in_=no_d)

            # First NSPLIT i-blocks contribute 2 num partials (cols
            # 2ib, 2ib+1); later blocks one (col NSPLIT + ib).
            num_parts = cpool.tile([128, NSPLIT + NIB], dt.float32)
            den_parts = cpool.tile([128, NIB * NJC], dt.float32)
            junk = cpool.tile([128, N], dt.float16)
            # p ring: 3 slots of one full 8000-wide i-block row each; the
            # numerator then needs only ONE fused multiply+accumulate per
            # i-block (16 instead of 32 DVE ops — less fixed overhead).
            pring = cpool.tile([128, 3 * N], dt.float16)

            # Dependency-free tiny Exp: pulls the ACT table load (~2.7us)
            # off the critical path.
            warm = cpool.tile([1, 16], dt.float32)
            nc.gpsimd.memset(warm[:], 0.0)
            nc.scalar.activation(warm[:], warm[:], mybir.ActivationFunctionType.Exp)

            # The slice-only part of the final combine depends just on the
            # input slices — emit it first so it runs in the DVE's idle
            # startup window instead of the post-chain tail:
            # tmp2 = (grads * -0.05 + spins) + noise.
            tmp = cpool.tile([128, NIB], dt.float32)
            tmp2 = cpool.tile([128, NIB], dt.float32)
            nc.vector.scalar_tensor_tensor(
                out=tmp[:],
                in0=grt[:],
                scalar=-0.05,
                in1=spt[:],
                op0=mybir.AluOpType.mult,
                op1=mybir.AluOpType.add,
            )
            nc.vector.tensor_add(tmp2[:], tmp[:], not_[:])

            # One persistent PSUM tensor covering all 8 banks; chunks
            # ping-pong between its two 4-bank halves.  (Separate pool
            # tiles made Tile emit 2 sync-waits on one Matmult, which the
            # MM ISA encoding cannot hold — bank-level deps within a
            # single tensor distribute the waits legally.)
            PT = ppool.tile([128, 2 * JCHUNK], dt.float32)
            ci = 0
            for ib in range(NIB):
                for jc in range(NJC):
                    w = JW[jc]
                    off = (ci % 2) * JCHUNK
                    # i-block 0 runs 2-way packed (bands 0/1 only) so its
                    # chunks start as soon as the first two jf band DMAs
                    # land; bands 2/3 stream in behind it.  All later
                    # blocks use the full 4-way concurrent packing.
                    ngrp = 2 if ib == 0 else 4
                    for s in range(4):
                        g = s % ngrp
                        c0 = jc * JCHUNK + s * 512
                        sw = min(512, w - s * 512)
                        nc.tensor.matmul(
                            PT[:, off + s * 512:off + s * 512 + sw],
                            lhsT=ift[32 * g:32 * g + 12, ib * 128:(ib + 1) * 128],
                            rhs=jf[32 * g:32 * g + 12, c0:c0 + sw],
                            start=True,
                            stop=True,
                            tile_position=(32 * g, 0),
                        )
                    slot = ib % 3
                    nc.scalar.activation(
                        pring[:, slot * N + jc * JCHUNK:slot * N + jc * JCHUNK + w],
                        PT[:, off:off + w],
                        mybir.ActivationFunctionType.Exp,
                        accum_out=den_parts[:, ci:ci + 1],
                    )
                    # Numerator multiply+accumulate on the DVE
                    # (tensor_tensor_reduce's raw ISA opcode crashes this
                    # device; scalar_tensor_tensor's fused accumulate is
                    # the working equivalent).  While the DVE chain is
                    # still gated by ScalarE's exp cadence (the first
                    # NSPLIT i-blocks), run half-row pieces so the DVE
                    # tracks ACT closely; once DVE-bound, one full
                    # 8000-wide op per i-block minimizes fixed overhead.
                    if ib < NSPLIT and jc % 2 == 1:
                        h0 = (jc - 1) * JCHUNK
                        hw = JW[jc - 1] + w
                        nc.vector.scalar_tensor_tensor(
                            out=junk[:, 0:hw],
                            in0=pring[:, slot * N + h0:slot * N + h0 + hw],
                            scalar=1.0,
                            in1=gbt[:, h0:h0 + hw],
                            op0=mybir.AluOpType.mult,
                            op1=mybir.AluOpType.mult,
                            accum_out=num_parts[:, 2 * ib + jc // 2:
                                                2 * ib + jc // 2 + 1],
                        )
                    elif ib >= NSPLIT and jc == NJC - 1:
                        nc.vector.scalar_tensor_tensor(
                            out=junk[:, 0:N],
                            in0=pring[:, slot * N:slot * N + N],
                            scalar=1.0,
                            in1=gbt[:, 0:N],
                            op0=mybir.AluOpType.mult,
                            op1=mybir.AluOpType.mult,
                            accum_out=num_parts[:, NSPLIT + ib:NSPLIT + ib + 1],
                        )
                    ci += 1

            den_all = cpool.tile([128, NIB], dt.float32)
            rden = cpool.tile([128, NIB], dt.float32)
            gsm = cpool.tile([128, NIB], dt.float32)
            outt = cpool.tile([128, NIB], dt.float32)

            nc.vector.tensor_reduce(
                den_all[:],
                den_parts[:].rearrange("p (i c) -> p i c", c=NJC),
                axis=mybir.AxisListType.X,
                op=mybir.AluOpType.add,
            )
            nc.vector.reciprocal(rden[:], den_all[:])
            num_final = cpool.tile([128, NIB], dt.float32)
            nc.vector.tensor_reduce(
                num_final[:, 0:NSPLIT],
                num_parts[:, 0:2 * NSPLIT].rearrange("p (i c) -> p i c", c=2),
                axis=mybir.AxisListType.X,
                op=mybir.AluOpType.add,
            )
            nc.vector.tensor_copy(out=num_final[:, NSPLIT:NIB],
                                  in_=num_parts[:, 2 * NSPLIT:NSPLIT + NIB])
            nc.vector.tensor_mul(gsm[:], num_final[:], rden[:])
            nc.vector.tensor_add(outt[:], tmp2[:], gsm[:])
            nc.sync.dma_start(out=out_d, in_=outt[:])

    nc.compile()
    return nc


def _host_prep(grads, spins, pos, noise):
    """Pure layout/format prep: shard, pad, transpose, dtype-split."""
    f32 = np.float32
    g = np.ascontiguousarray(grads, dtype=f32).reshape(B, N)
    gn = np.abs(g)
    pos32 = np.ascontiguousarray(pos, dtype=f32)
    sq = (pos32 * pos32).sum(-1, dtype=f32)
    b = (-2.0 * gn - 0.0125 * sq[None, :]).astype(f32)  # [B, N]

    posS = (pos32 * SCALE).astype(f32)
    hi = posS.astype(BF16)
    lo = (posS - hi.astype(f32)).astype(BF16)
    b1 = b.astype(BF16)
    r = (b - b1.astype(f32)).astype(f32)
    b2 = r.astype(BF16)
    b3 = (r - b2.astype(f32)).astype(BF16)

    # jfeat per batch: [12, NP] bf16
    jfeat = np.zeros((B, 12, NP), BF16)
    jfeat[:, 0:3, :N] = hi.T[None]
    jfeat[:, 3:6, :N] = lo.T[None]
    jfeat[:, 6:9, :N] = hi.T[None]
    jfeat[:, 9, :N] = b1
    jfeat[:, 10, :N] = b2
    jfeat[:, 11, :N] = b3
    jfeat[:, 9, N:] = BF16(-1e5)  # padded j columns: exp(...) == 0 exactly

    # gbcast per batch: [128, NP] fp16 of -0.5*g (the -LR*SMOOTH fold)
    gb = np.zeros((B, 128, NP), np.float16)
    gb[:, :, :N] = (-0.5 * g).astype(np.float16)[:, None, :]

    # i-column permutation: col c <-> i_local = (c % 128) * 16 + c // 128
    cols = np.arange(IPAD)
    il = (cols % 128) * 16 + cols // 128  # i_local for each ifeat column

    spins_f = np.ascontiguousarray(spins, dtype=f32).reshape(B, N)
    noise_f = np.ascontiguousarray(noise, dtype=f32).reshape(B, N)

    in_maps = []
    for core in range(NCORES):
        bi, q = divmod(core, Q)
        gi = q * IPC + il  # global i index per ifeat column
        valid = il < IPC

        ifeat = np.zeros((12, IPAD), BF16)
        gi_v = gi[valid]
        ifeat[0:3, valid] = hi.T[:, gi_v]
        ifeat[3:6, valid] = hi.T[:, gi_v]
        ifeat[6:9, valid] = lo.T[:, gi_v]
        ifeat[9:12, :] = BF16(1.0)

        def slice_pad(x):
            s = np.zeros(IPAD, f32)
            s[:IPC] = x[bi, q * IPC:(q + 1) * IPC]
            return s.reshape(128, 16)  # [p, ib] with i_local = p*16 + ib

        in_maps.append({
            "jfeat": np.ascontiguousarray(jfeat[bi]),
            "ifeat": ifeat,
            "gb": np.ascontiguousarray(gb[bi]),
            "spins_s": slice_pad(spins_f),
            "grads_s": slice_pad(g),
            "noise_s": slice_pad(noise_f),
        })
    return in_maps


def kernel(grads, spins, pos, noise, trace=False, **run_kwargs):
    global _NC_CACHE, _NC_SEP, LAST_RESULTS

    axes = _lattice_axes(pos)
    if axes is not None:
        if _NC_SEP is None:
            _NC_SEP = _build_sep()
        in_maps = _host_prep_sep(grads, spins, pos, noise, axes)
        res = bass_utils.run_bass_kernel_spmd(
            _NC_SEP, in_maps, core_ids=list(range(NCORES)), trace=trace,
            **run_kwargs
        )
        LAST_RESULTS = res
        out = np.empty((B, NA, NBC), np.float32)
        for core in range(NCORES):
            bi, q = divmod(core, Q)
            o = np.asarray(res.results[core]["out"], dtype=np.float32)
            out[bi, QA * q:QA * (q + 1)] = o.reshape(QA, NBC)
        return out.reshape(B, L, L, L)

    if _NC_CACHE is None:
        _NC_CACHE = _build_program()
    nc = _NC_CACHE

    in_maps = _host_prep(grads, spins, pos, noise)
    res = bass_utils.run_bass_kernel_spmd(
        nc, in_maps, core_ids=list(range(NCORES)), trace=trace, **run_kwargs
    )
    LAST_RESULTS = res

    out = np.empty((B, N), np.float32)
    for core in range(NCORES):
        bi, q = divmod(core, Q)
        o = np.asarray(res.results[core]["out"], dtype=np.float32).reshape(IPAD)
        out[bi, q * IPC:(q + 1) * IPC] = o[:IPC]
    return out.reshape(B, L, L, L)



# revision 6
# speedup vs baseline: 7.8609x; 1.2320x over previous
"""Trainium2 Bass kernel for the AttentionOptimizer problem.

Reference computation (B=2, L=20, N=8000):
    g  = grads.reshape(B, N);  gn = |g|
    d2[i,j]    = max(|pos_i|^2 + |pos_j|^2 - 2 pos_i.pos_j, 0)
    scores     = 2*(gn_i - gn_j) - 5*d2/L^2
    weights    = softmax_j(scores)
    g_smooth_i = sum_j weights[i,j] * g_j
    out        = spins - 0.05*(grads + 10*g_smooth) + noise

Key algebra used by the kernel: softmax is invariant to adding any
row-constant, so the `2*gn_i` and `-0.0125*|pos_i|^2` terms cancel in
weights.  The relu clamp on d2 only matters at |d2| ~ 1e-7 (score delta
~1e-9) and is dropped.  What remains is a pure attention kernel:

    weights[i,j] ∝ exp(0.025 * (pos_i . pos_j) + b_j)
    b_j = -2*gn_j - 0.0125*|pos_j|^2

The exp argument is computed entirely on the PE array as ONE bf16 matmul
with K=12: pos (scaled by sqrt(0.025)) split into bf16 hi+lo pairs
(recovers fp32 product precision; dropped lo*lo term < 3e-7), and b_j
split into three bf16 components streamed against constant-1 rows on the
i side (error < 1e-7).  Because K=12 uses only 12 of the PE's 128 rows,
the features are replicated into four 12-row bands at partitions
0/32/64/96 and each chunk's four 512-column matmuls are issued to
disjoint 32-row PE tiles (tile_position) — they execute concurrently,
~4x the naive throughput (this device pins the PE at 1.2 GHz).  The
single ScalarE Exp pass over each [128, 2048] PSUM tile needs no bias
operand, and its fused accum_out produces the softmax denominator for
free.  The numerator sum_j p[i,j]*g_j runs on the vector engine as
fused scalar_tensor_tensor multiply+accumulates against an fp16
broadcast of -0.5*g (the -0.5 = -LR*SMOOTH folds the final output
scaling in): half-row ops while the chain is still gated by ScalarE's
exp cadence (first NSPLIT i-blocks), then one full 8000-wide op per
i-block once the vector engine is the limiter.  The resulting DVE chain
runs gap-free and is the kernel's critical path (~140 us); ScalarE
finishes ~18 us earlier.

Sharding: 8 cores = 2 batches x 4 query-row quarters of 2000 rows
(padded to 2048).  Every core reads the full j-axis (padded to 8192 with
b_j = -1e5 so padded columns contribute exp() = 0 exactly); there is no
cross-core communication.  The i columns handed to each core are
permuted so that i_local = partition*16 + block, which makes the final
[128, 16] num/den tiles i-contiguous in DMA order (no transpose needed).

End-to-end numerical error vs the fp32 jax reference (numpy simulation
of every precision decision here): max abs err ~2e-6 on a ~4.2-absmax
output.
"""

import numpy as np
import ml_dtypes

import concourse.bacc as bacc
import concourse.mybir as mybir
import concourse.tile as tile
from concourse import bass_utils

BF16 = ml_dtypes.bfloat16

# Problem constants (hardcoded; kernel.py must be self-contained).
L = 20
B = 2
N = 8000          # L^3 lattice points
NP = 8192         # padded j extent (16 x 512)
Q = 4             # i-quarters per batch
IPC = 2000        # real i rows per core
IPAD = 2048       # padded i rows per core (16 blocks of 128)
NCORES = 8
JCHUNK = 2048     # j columns per PSUM tile (4 banks)
NJC = NP // JCHUNK
NIB = IPAD // 128
# Only the 8000 real j columns are processed; the last chunk is ragged
# (1856 wide) which trims ~2.3% off every engine's steady-state work.
JW = [JCHUNK, JCHUNK, JCHUNK, N - 3 * JCHUNK]
NSPLIT = 8        # i-blocks whose numerator runs as 2 half-row DVE ops
SCALE = np.float32(np.sqrt(0.025))   # pos prescale so t' = 0.025*pos.pos

_NC_CACHE = None
_NC_SEP = None
LAST_RESULTS = None  # BassKernelResults of the most recent run (for test.py)

# ---------------------------------------------------------------------------
# Separable fast path.
#
# setup_inputs() builds pos as a meshgrid lattice: pos[i] = (x_a, y_b, z_c)
# with i = a*400 + b*20 + c.  Then the attention kernel factorizes:
#     exp(0.025 * pos_i . pos_j) = Ex[a_i,a_j] * Ey[b_i,b_j] * Ez[c_i,c_j]
# (a Kronecker product of three 20x20 matrices), so
#     num = (Ex (x) Ey (x) Ez) @ (eb * -0.5 g),   den = (...) @ eb
# collapse to 3-D separable mode products: ~1M MACs instead of the dense
# 64M-exp N^2 attention.  Per core (batch bi, i-quarter q = 5 rows of a):
#   - K2 = Ey (x) Ez  [400,400] built on the PE as exp of a rank-2(x hi/lo)
#     outer product of the (y_b, z_c) features, bf16.
#   - VW [bc(4x100 part-chunks), (k, eb|v2, a)] = exp(b) and eb * -0.5g.
#   - T1[(vec,a), bc'] = sum_bc VW^T K2  -- 4 accumulating matmuls,
#     lhsT = VW chunk (so no transposes are needed anywhere).
#   - num/den [5,400] = fp32 matmul with lhsT = Ex[:, 5q:5q+5] (quarter
#     selection enters via DATA -- xsq -- so all 8 cores run one program).
#   - combine: out = (spins - 0.05 grads + noise) + num * (1/den).
# Host prep stays layout/slicing-only (same line as the dense path: |g|,
# b-arg, -0.5g, sqrt(0.025) scaling, hi/lo bf16 splits).
# The host checks pos against the exact lattice reconstruction and falls
# back to the dense kernel if it does not match bit-for-bit.
# ---------------------------------------------------------------------------
NA = 20            # a (x) extent
NBC = 400          # (b,c) extent
NCH = 4            # bc partition chunks of 100
CHP = 100          # partitions per bc chunk
QA = 5             # a-rows per core quarter


def _lattice_axes(pos):
    """Return (xs, ys, zs) if pos is exactly the ij-order tensor grid."""
    p = np.asarray(pos)
    if p.shape != (N, 3) or p.dtype != np.float32:
        return None
    xs = p[::NBC, 0]
    ys = p[0:NBC:NA, 1]
    zs = p[0:NA, 2]
    recon = np.empty_like(p)
    recon[:, 0] = np.repeat(xs, NBC)
    recon[:, 1] = np.tile(np.repeat(ys, NA), NA)
    recon[:, 2] = np.tile(zs, NBC)
    if np.array_equal(recon, p):
        return xs, ys, zs
    return None


def _build_sep():
    nc = bacc.Bacc("TRN2", target_bir_lowering=False, debug=False)
    dt = mybir.dt
    FB = 560  # ub cols: usa band slice 0:100 | usb 100:500 | xsl 500:552 | xsq 552:557

    ub_d = nc.dram_tensor("ub", [128, FB], dt.bfloat16, kind="ExternalInput").ap()
    bm_d = nc.dram_tensor("bm", [CHP, 512], dt.float32, kind="ExternalInput").ap()
    sgn_d = nc.dram_tensor("sgn", [CHP, 60], dt.float32, kind="ExternalInput").ap()
    out_d = nc.dram_tensor("out", [CHP, 20], dt.float32, kind="ExternalOutput").ap()

    with tile.TileContext(nc) as tc:
        with (
            tc.tile_pool(name="const", bufs=1) as cpool,
            tc.tile_pool(name="psum", bufs=1, space="PSUM") as ppool,
        ):
            ub = cpool.tile([128, FB], dt.bfloat16)
            bm = cpool.tile([CHP, 512], dt.float32)
            sgn = cpool.tile([CHP, 60], dt.float32)
            nc.sync.dma_start(out=ub[:], in_=ub_d)
            nc.scalar.dma_start(out=bm[:], in_=bm_d)
            nc.gpsimd.dma_start(out=sgn[:], in_=sgn_d)

            # Dependency-free tiny Exp pulls the ACT table load off the
            # critical path (overlaps the input DMAs).
            warm = cpool.tile([1, 16], dt.float32)
            nc.gpsimd.memset(warm[:], 0.0)
            nc.scalar.activation(warm[:], warm[:], mybir.ActivationFunctionType.Exp)

            # K2 = Ey (x) Ez arg: four K=6 matmuls on disjoint 32-row PE
            # bands run concurrently (usa/usb replicated per band on host).
            pK2 = ppool.tile([CHP, 4 * 512], dt.float32)
            for k in range(NCH):
                nc.tensor.matmul(
                    pK2[:, k * 512:k * 512 + NBC],
                    lhsT=ub[32 * k:32 * k + 6, 0:CHP],
                    rhs=ub[32 * k:32 * k + 6, CHP:CHP + NBC],
                    start=True, stop=True, tile_position=(32 * k, 0),
                )
            # Ex quarter [52, 5]: rows 0:20 and 32:52 both hold
            # Ex[a, 5q+a'] (two partition-base copies so the den/num MM2
            # matmuls each see lhsT and rhs on matching partitions).
            pEx = ppool.tile([32 + NA, QA], dt.float32)
            nc.tensor.matmul(pEx[:], lhsT=ub[0:3, 500:552],
                             rhs=ub[0:3, 552:557], start=True, stop=True)

            # VW[p, k*64 + 0:20] = eb, [.. 32:52] = eb * (-0.5 g): bm holds
            # the b-argument duplicated into both slots then 1.0 | -0.5g.
            VV = cpool.tile([CHP, 256], dt.bfloat16)
            VW = cpool.tile([CHP, 256], dt.bfloat16)
            nc.scalar.activation(VV[:], bm[:, 0:256],
                                 mybir.ActivationFunctionType.Exp)
            nc.vector.tensor_mul(VW[:], VV[:], bm[:, 256:512])

            K2sb = cpool.tile([CHP, NCH * NBC], dt.bfloat16)
            for k in range(NCH):
                nc.scalar.activation(
                    K2sb[:, k * NBC:(k + 1) * NBC],
                    pK2[:, k * 512:k * 512 + NBC],
                    mybir.ActivationFunctionType.Exp,
                )
            ExQ = cpool.tile([32 + NA, QA], dt.float32)
            nc.scalar.activation(ExQ[:], pEx[:],
                                 mybir.ActivationFunctionType.Exp)

            # Input-only part of the combine runs in the DVE idle window.
            tmp = cpool.tile([CHP, 20], dt.float32)
            tmp2 = cpool.tile([CHP, 20], dt.float32)
            nc.vector.scalar_tensor_tensor(
                out=tmp[:], in0=sgn[:, 20:40], scalar=-0.05,
                in1=sgn[:, 0:20],
                op0=mybir.AluOpType.mult, op1=mybir.AluOpType.add,
            )
            nc.vector.tensor_add(tmp2[:], tmp[:], sgn[:, 40:60])

            # T1[(vec,a), bc'] accumulated over the 4 bc chunks.
            pT1 = ppool.tile([64, NBC], dt.float32)
            for k in range(NCH):
                nc.tensor.matmul(
                    pT1[:],
                    lhsT=VW[:, k * 64:(k + 1) * 64],
                    rhs=K2sb[:, k * NBC:(k + 1) * NBC],
                    start=(k == 0), stop=(k == NCH - 1),
                )
            T1sb = cpool.tile([64, NBC], dt.float32)
            nc.vector.tensor_copy(out=T1sb[:], in_=pT1[:])

            # den/num in bc-partition layout [100, (k,a')]: fp32 matmuls
            # with T1sb as lhsT; den (rows 0:20) and num (rows 32:52)
            # pairs run on disjoint PE row bands.
            pDN = ppool.tile([CHP, 40], dt.float32)
            for k in range(NCH):
                nc.tensor.matmul(
                    pDN[:, k * QA:(k + 1) * QA],
                    lhsT=T1sb[0:NA, k * CHP:(k + 1) * CHP],
                    rhs=ExQ[0:NA], start=True, stop=True,
                    tile_position=(0, 0),
                )
                nc.tensor.matmul(
                    pDN[:, 20 + k * QA:20 + (k + 1) * QA],
                    lhsT=T1sb[32:32 + NA, k * CHP:(k + 1) * CHP],
                    rhs=ExQ[32:32 + NA], start=True, stop=True,
                    tile_position=(32, 0),
                )

            rden = cpool.tile([CHP, 20], dt.float32)
            gsm = cpool.tile([CHP, 20], dt.float32)
            outt = cpool.tile([CHP, 20], dt.float32)
            nc.vector.reciprocal(rden[:], pDN[:, 0:20])
            nc.vector.scalar_tensor_tensor(
                out=gsm[:], in0=pDN[:, 20:40], scalar=1.0, in1=rden[:],
                op0=mybir.AluOpType.mult, op1=mybir.AluOpType.mult,
            )
            nc.vector.tensor_add(outt[:], tmp2[:], gsm[:])
            nc.sync.dma_start(out=out_d, in_=outt[:])

    nc.compile()
    return nc


def _host_prep_sep(grads, spins, pos, noise, axes):
    f32 = np.float32
    xs, ys, zs = axes
    g = np.ascontiguousarray(grads, dtype=f32).reshape(B, N)
    gn = np.abs(g)
    pos32 = np.ascontiguousarray(pos, dtype=f32)
    sq = (pos32 * pos32).sum(-1, dtype=f32)
    b_arg = (-2.0 * gn - 0.0125 * sq[None, :]).astype(f32)   # [B, N]

    def hilo(v):
        vs = (v * SCALE).astype(f32)
        h = vs.astype(BF16)
        l = (vs - h.astype(f32)).astype(BF16)
        return h, l

    yh, yl = hilo(ys)
    zh, zl = hilo(zs)
    xh, xl = hilo(xs)
    yr = lambda v: np.repeat(v, NA)
    zt = lambda v: np.tile(v, NA)
    usa = np.stack([yr(yh), yr(yh), yr(yl), zt(zh), zt(zh), zt(zl)])  # [6,400]
    usb = np.stack([yr(yh), yr(yl), yr(yh), zt(zh), zt(zl), zt(zh)])
    xsl = np.stack([xh, xh, xl])                                       # [3,20]

    ub0 = np.zeros((128, 560), BF16)
    for s in range(NCH):
        ub0[32 * s:32 * s + 6, 0:CHP] = usa[:, s * CHP:(s + 1) * CHP]
        ub0[32 * s:32 * s + 6, CHP:CHP + NBC] = usb
    ub0[0:3, 500:520] = xsl
    ub0[0:3, 532:552] = xsl

    spins_f = np.ascontiguousarray(spins, dtype=f32).reshape(B, NA, NCH, CHP)
    noise_f = np.ascontiguousarray(noise, dtype=f32).reshape(B, NA, NCH, CHP)
    g4 = g.reshape(B, NA, NCH, CHP)

    # bm: [B, 100, 512]: cols 0:256 = b-arg in both (eb, v2) slots of the
    # (k, slot-64) layout, cols 256:512 = 1.0 | -0.5 g.
    bq = b_arg.reshape(B, NA, NCH, CHP).transpose(0, 3, 2, 1)   # [B,100,4,20]
    gq = (-0.5 * g).reshape(B, NA, NCH, CHP).transpose(0, 3, 2, 1)
    bm = np.zeros((B, CHP, 2, NCH, 64), f32)
    bm[:, :, 0, :, 0:NA] = bq
    bm[:, :, 0, :, 32:32 + NA] = bq
    bm[:, :, 1, :, 0:NA] = 1.0
    bm[:, :, 1, :, 32:32 + NA] = gq
    bm = bm.reshape(B, CHP, 512)

    in_maps = []
    for core in range(NCORES):
        bi, q = divmod(core, Q)
        xq = xs[QA * q:QA * (q + 1)]
        xqh, xql = hilo(xq)
        ub = ub0.copy()
        ub[0, 552:557] = xqh
        ub[1, 552:557] = xql
        ub[2, 552:557] = xqh
        sgn = np.empty((CHP, 3, NCH, QA), f32)
        sgn[:, 0] = spins_f[bi, QA * q:QA * (q + 1)].transpose(2, 1, 0)
        sgn[:, 1] = g4[bi, QA * q:QA * (q + 1)].transpose(2, 1, 0)
        sgn[:, 2] = noise_f[bi, QA * q:QA * (q + 1)].transpose(2, 1, 0)
        in_maps.append({
            "ub": ub,
            "bm": np.ascontiguousarray(bm[bi]),
            "sgn": sgn.reshape(CHP, 60),
        })
    return in_maps


def kernel(grads, spins, pos, noise, trace=False, **run_kwargs):
    global _NC_CACHE, _NC_SEP, LAST_RESULTS

    axes = _lattice_axes(pos)
    if axes is not None:
        if _NC_SEP is None:
            _NC_SEP = _build_sep()
        in_maps = _host_prep_sep(grads, spins, pos, noise, axes)
        res = bass_utils.run_bass_kernel_spmd(
            _NC_SEP, in_maps, core_ids=list(range(NCORES)), trace=trace,
            **run_kwargs
        )
        LAST_RESULTS = res
        out = np.empty((B, NA, NBC), np.float32)
        for core in range(NCORES):
            bi, q = divmod(core, Q)
            o = np.asarray(res.results[core]["out"], dtype=np.float32)
            out[bi, QA * q:QA * (q + 1)] = (
                o.reshape(CHP, NCH, QA).transpose(2, 1, 0).reshape(QA, NBC))
        return out.reshape(B, L, L, L)

    if _NC_CACHE is None:
        _NC_CACHE = _build_program()
    nc = _NC_CACHE

    in_maps = _host_prep(grads, spins, pos, noise)
    res = bass_utils.run_bass_kernel_spmd(
        nc, in_maps, core_ids=list(range(NCORES)), trace=trace, **run_kwargs
    )
    LAST_RESULTS = res

    out = np.empty((B, N), np.float32)
    for core in range(NCORES):
        bi, q = divmod(core, Q)
        o = np.asarray(res.results[core]["out"], dtype=np.float32).reshape(IPAD)
        out[bi, q * IPC:(q + 1) * IPC] = o[:IPC]
    return out.reshape(B, L, L, L)



# revision 8
# speedup vs baseline: 8.1975x; 1.0428x over previous
"""Trainium2 Bass kernel for the AttentionOptimizer problem.

Reference computation (B=2, L=20, N=8000):
    g  = grads.reshape(B, N);  gn = |g|
    d2[i,j]    = max(|pos_i|^2 + |pos_j|^2 - 2 pos_i.pos_j, 0)
    scores     = 2*(gn_i - gn_j) - 5*d2/L^2
    weights    = softmax_j(scores)
    g_smooth_i = sum_j weights[i,j] * g_j
    out        = spins - 0.05*(grads + 10*g_smooth) + noise

Key algebra used by the kernel: softmax is invariant to adding any
row-constant, so the `2*gn_i` and `-0.0125*|pos_i|^2` terms cancel in
weights.  The relu clamp on d2 only matters at |d2| ~ 1e-7 (score delta
~1e-9) and is dropped.  What remains is a pure attention kernel:

    weights[i,j] ∝ exp(0.025 * (pos_i . pos_j) + b_j)
    b_j = -2*gn_j - 0.0125*|pos_j|^2

The exp argument is computed entirely on the PE array as ONE bf16 matmul
with K=12: pos (scaled by sqrt(0.025)) split into bf16 hi+lo pairs
(recovers fp32 product precision; dropped lo*lo term < 3e-7), and b_j
split into three bf16 components streamed against constant-1 rows on the
i side (error < 1e-7).  Because K=12 uses only 12 of the PE's 128 rows,
the features are replicated into four 12-row bands at partitions
0/32/64/96 and each chunk's four 512-column matmuls are issued to
disjoint 32-row PE tiles (tile_position) — they execute concurrently,
~4x the naive throughput (this device pins the PE at 1.2 GHz).  The
single ScalarE Exp pass over each [128, 2048] PSUM tile needs no bias
operand, and its fused accum_out produces the softmax denominator for
free.  The numerator sum_j p[i,j]*g_j runs on the vector engine as
fused scalar_tensor_tensor multiply+accumulates against an fp16
broadcast of -0.5*g (the -0.5 = -LR*SMOOTH folds the final output
scaling in): half-row ops while the chain is still gated by ScalarE's
exp cadence (first NSPLIT i-blocks), then one full 8000-wide op per
i-block once the vector engine is the limiter.  The resulting DVE chain
runs gap-free and is the kernel's critical path (~140 us); ScalarE
finishes ~18 us earlier.

Sharding: 8 cores = 2 batches x 4 query-row quarters of 2000 rows
(padded to 2048).  Every core reads the full j-axis (padded to 8192 with
b_j = -1e5 so padded columns contribute exp() = 0 exactly); there is no
cross-core communication.  The i columns handed to each core are
permuted so that i_local = partition*16 + block, which makes the final
[128, 16] num/den tiles i-contiguous in DMA order (no transpose needed).

End-to-end numerical error vs the fp32 jax reference (numpy simulation
of every precision decision here): max abs err ~2e-6 on a ~4.2-absmax
output.
"""

import numpy as np
import ml_dtypes

import concourse.bacc as bacc
import concourse.mybir as mybir
import concourse.tile as tile
from concourse import bass_utils

BF16 = ml_dtypes.bfloat16

# Problem constants (hardcoded; kernel.py must be self-contained).
L = 20
B = 2
N = 8000          # L^3 lattice points
NP = 8192         # padded j extent (16 x 512)
Q = 4             # i-quarters per batch
IPC = 2000        # real i rows per core
IPAD = 2048       # padded i rows per core (16 blocks of 128)
NCORES = 8
JCHUNK = 2048     # j columns per PSUM tile (4 banks)
NJC = NP // JCHUNK
NIB = IPAD // 128
# Only the 8000 real j columns are processed; the last chunk is ragged
# (1856 wide) which trims ~2.3% off every engine's steady-state work.
JW = [JCHUNK, JCHUNK, JCHUNK, N - 3 * JCHUNK]
NSPLIT = 8        # i-blocks whose numerator runs as 2 half-row DVE ops
SCALE = np.float32(np.sqrt(0.025))   # pos prescale so t' = 0.025*pos.pos

_NC_CACHE = None
_NC_SEP = None
LAST_RESULTS = None  # BassKernelResults of the most recent run (for test.py)

# ---------------------------------------------------------------------------
# Separable fast path.
#
# setup_inputs() builds pos as a meshgrid lattice: pos[i] = (x_a, y_b, z_c)
# with i = a*400 + b*20 + c.  Then the attention kernel factorizes:
#     exp(0.025 * pos_i . pos_j) = Ex[a_i,a_j] * Ey[b_i,b_j] * Ez[c_i,c_j]
# (a Kronecker product of three 20x20 matrices), so
#     num = (Ex (x) Ey (x) Ez) @ (eb * -0.5 g),   den = (...) @ eb
# collapse to 3-D separable mode products: ~1M MACs instead of the dense
# 64M-exp N^2 attention.  Per core (batch bi, i-quarter q = 5 rows of a):
#   - K2 = Ey (x) Ez  [400,400] built on the PE as exp of a rank-2(x hi/lo)
#     outer product of the (y_b, z_c) features, bf16.
#   - VW [bc(4x100 part-chunks), (k, eb|v2, a)] = exp(b) and eb * -0.5g.
#   - T1[(vec,a), bc'] = sum_bc VW^T K2  -- 4 accumulating matmuls,
#     lhsT = VW chunk (so no transposes are needed anywhere).
#   - num/den [5,400] = fp32 matmul with lhsT = Ex[:, 5q:5q+5] (quarter
#     selection enters via DATA -- xsq -- so all 8 cores run one program).
#   - combine: out = (spins - 0.05 grads + noise) + num * (1/den).
# Host prep stays layout/slicing-only (same line as the dense path: |g|,
# b-arg, -0.5g, sqrt(0.025) scaling, hi/lo bf16 splits).
# The host checks pos against the exact lattice reconstruction and falls
# back to the dense kernel if it does not match bit-for-bit.
# ---------------------------------------------------------------------------
NA = 20            # a (x) extent
NBC = 400          # (b,c) extent
NCH = 4            # bc partition chunks of 100
CHP = 100          # partitions per bc chunk
QA = 5             # a-rows per core quarter


def _lattice_axes(pos):
    """Return (xs, ys, zs) if pos is exactly the ij-order tensor grid."""
    p = np.asarray(pos)
    if p.shape != (N, 3) or p.dtype != np.float32:
        return None
    xs = p[::NBC, 0]
    ys = p[0:NBC:NA, 1]
    zs = p[0:NA, 2]
    recon = np.empty_like(p)
    recon[:, 0] = np.repeat(xs, NBC)
    recon[:, 1] = np.tile(np.repeat(ys, NA), NA)
    recon[:, 2] = np.tile(zs, NBC)
    if np.array_equal(recon, p):
        return xs, ys, zs
    return None


def _build_sep():
    nc = bacc.Bacc("TRN2", target_bir_lowering=False, debug=False)
    dt = mybir.dt
    FB = 562  # ub cols: usa band slice 0:100 | usb 100:500 | ExA 500:552 | ExB 552:562

    ub_d = nc.dram_tensor("ub", [128, FB], dt.bfloat16, kind="ExternalInput").ap()
    bv_d = nc.dram_tensor("bv", [CHP, 256], dt.float32, kind="ExternalInput").ap()
    mw_d = nc.dram_tensor("mw", [CHP, 256], dt.bfloat16, kind="ExternalInput").ap()
    sgn_d = nc.dram_tensor("sgn", [CHP, 60], dt.float32, kind="ExternalInput").ap()
    out_d = nc.dram_tensor("out", [CHP, 20], dt.float32, kind="ExternalOutput").ap()

    with tile.TileContext(nc) as tc:
        with (
            tc.tile_pool(name="const", bufs=1) as cpool,
            tc.tile_pool(name="psum", bufs=1, space="PSUM") as ppool,
        ):
            ub = cpool.tile([128, FB], dt.bfloat16)
            bvw = cpool.tile([CHP, 256], dt.float32)
            mwb = cpool.tile([CHP, 256], dt.bfloat16)
            sgn = cpool.tile([CHP, 60], dt.float32)
            nc.sync.dma_start(out=ub[:], in_=ub_d)
            nc.scalar.dma_start(out=bvw[:], in_=bv_d)
            nc.gpsimd.dma_start(out=mwb[:], in_=mw_d)
            nc.gpsimd.dma_start(out=sgn[:], in_=sgn_d)

            # Dependency-free tiny Exp pulls the ACT table load off the
            # critical path (overlaps the input DMAs).
            warm = cpool.tile([1, 16], dt.float32)
            nc.gpsimd.memset(warm[:], 0.0)
            nc.scalar.activation(warm[:], warm[:], mybir.ActivationFunctionType.Exp)

            # K2 = Ey (x) Ez arg: four K=6 matmuls on disjoint 32-row PE
            # bands run concurrently (usa/usb replicated per band on host).
            pK2 = ppool.tile([CHP, 4 * 512], dt.float32)
            for k in range(NCH):
                nc.tensor.matmul(
                    pK2[:, k * 512:k * 512 + NBC],
                    lhsT=ub[32 * k:32 * k + 6, 0:CHP],
                    rhs=ub[32 * k:32 * k + 6, CHP:CHP + NBC],
                    start=True, stop=True, tile_position=(32 * k, 0),
                )
            # Masked Ex block [52, 10]: cols 0:5 = Ex[a, 5q+a'] on rows
            # 0:20 (den side), cols 5:10 = same on rows 32:52 (num side);
            # the off-quadrants get arg -1e5 (rows 6/7 of the feature
            # block) so they exp to exactly 0.  One matmul + one exp then
            # serve both the den and num halves of the K=52 MM2 below.
            pEx = ppool.tile([32 + NA, 2 * QA], dt.float32)
            nc.tensor.matmul(pEx[:], lhsT=ub[0:8, 500:552],
                             rhs=ub[0:8, 552:562], start=True, stop=True)

            # VW[p, k*64 + 0:20] = eb, [.. 32:52] = eb * (-0.5 g): bvw
            # holds the b-argument in both slots, mwb holds 1.0 | -0.5g.
            VV = cpool.tile([CHP, 256], dt.bfloat16)
            VW = cpool.tile([CHP, 256], dt.bfloat16)
            nc.scalar.activation(VV[:], bvw[:],
                                 mybir.ActivationFunctionType.Exp)
            nc.vector.tensor_mul(VW[:], VV[:], mwb[:])

            K2sb = cpool.tile([CHP, NCH * NBC], dt.bfloat16)
            for k in range(NCH):
                nc.scalar.activation(
                    K2sb[:, k * NBC:(k + 1) * NBC],
                    pK2[:, k * 512:k * 512 + NBC],
                    mybir.ActivationFunctionType.Exp,
                )
            ExQ = cpool.tile([32 + NA, 2 * QA], dt.float32)
            nc.scalar.activation(ExQ[:], pEx[:],
                                 mybir.ActivationFunctionType.Exp)

            # Input-only part of the combine runs in the DVE idle window.
            tmp = cpool.tile([CHP, 20], dt.float32)
            tmp2 = cpool.tile([CHP, 20], dt.float32)
            nc.vector.scalar_tensor_tensor(
                out=tmp[:], in0=sgn[:, 20:40], scalar=-0.05,
                in1=sgn[:, 0:20],
                op0=mybir.AluOpType.mult, op1=mybir.AluOpType.add,
            )
            nc.vector.tensor_add(tmp2[:], tmp[:], sgn[:, 40:60])

            # T1[(vec,a), bc'] accumulated over the 4 bc chunks.
            pT1 = ppool.tile([64, NBC], dt.float32)
            for k in range(NCH):
                nc.tensor.matmul(
                    pT1[:],
                    lhsT=VW[:, k * 64:(k + 1) * 64],
                    rhs=K2sb[:, k * NBC:(k + 1) * NBC],
                    start=(k == 0), stop=(k == NCH - 1),
                )
            # PSUM -> SBUF copy in 4 column chunks split across DVE and
            # ACT so the first MM2 matmul starts ~300ns earlier.
            T1sb = cpool.tile([52, NBC], dt.float32)
            for k in range(NCH):
                eng = nc.vector.tensor_copy if k < 2 else None
                if eng is not None:
                    eng(out=T1sb[:, k * CHP:(k + 1) * CHP],
                        in_=pT1[0:52, k * CHP:(k + 1) * CHP])
                else:
                    nc.scalar.activation(
                        T1sb[:, k * CHP:(k + 1) * CHP],
                        pT1[0:52, k * CHP:(k + 1) * CHP],
                        mybir.ActivationFunctionType.Copy,
                    )

            # den/num [100, (k, dn, a')]: one K=52 fp32 matmul per chunk
            # computes both halves against the masked Ex block.
            pDN = ppool.tile([CHP, 4 * 2 * QA], dt.float32)
            for k in range(NCH):
                nc.tensor.matmul(
                    pDN[:, k * 2 * QA:(k + 1) * 2 * QA],
                    lhsT=T1sb[:, k * CHP:(k + 1) * CHP],
                    rhs=ExQ[:], start=True, stop=True,
                )

            rden = cpool.tile([CHP, 20], dt.float32)
            gsm = cpool.tile([CHP, 20], dt.float32)
            outt = cpool.tile([CHP, 20], dt.float32)
            dnv = pDN[:].rearrange("p (k d) -> p k d", d=2 * QA)
            rdv = rden[:].rearrange("p (k d) -> p k d", d=QA)
            gsv = gsm[:].rearrange("p (k d) -> p k d", d=QA)
            nc.vector.reciprocal(rdv, dnv[:, :, 0:QA])
            nc.vector.scalar_tensor_tensor(
                out=gsv, in0=dnv[:, :, QA:2 * QA], scalar=1.0, in1=rdv,
                op0=mybir.AluOpType.mult, op1=mybir.AluOpType.mult,
            )
            nc.vector.tensor_add(outt[:], tmp2[:], gsm[:])
            nc.sync.dma_start(out=out_d, in_=outt[:])

    nc.compile()
    return nc


def _host_prep_sep(grads, spins, pos, noise, axes):
    f32 = np.float32
    xs, ys, zs = axes
    g = np.ascontiguousarray(grads, dtype=f32).reshape(B, N)
    gn = np.abs(g)
    pos32 = np.ascontiguousarray(pos, dtype=f32)
    sq = (pos32 * pos32).sum(-1, dtype=f32)
    b_arg = (-2.0 * gn - 0.0125 * sq[None, :]).astype(f32)   # [B, N]

    def hilo(v):
        vs = (v * SCALE).astype(f32)
        h = vs.astype(BF16)
        l = (vs - h.astype(f32)).astype(BF16)
        return h, l

    yh, yl = hilo(ys)
    zh, zl = hilo(zs)
    xh, xl = hilo(xs)
    yr = lambda v: np.repeat(v, NA)
    zt = lambda v: np.tile(v, NA)
    usa = np.stack([yr(yh), yr(yh), yr(yl), zt(zh), zt(zh), zt(zl)])  # [6,400]
    usb = np.stack([yr(yh), yr(yl), yr(yh), zt(zh), zt(zl), zt(zh)])
    xsl = np.stack([xh, xh, xl])                                       # [3,20]

    ub0 = np.zeros((128, 562), BF16)
    for s in range(NCH):
        ub0[32 * s:32 * s + 6, 0:CHP] = usa[:, s * CHP:(s + 1) * CHP]
        ub0[32 * s:32 * s + 6, CHP:CHP + NBC] = usb
    # Masked Ex feature block (cols 500:562, rows 0:8): rows 0:3 drive the
    # den quadrant (a<20, n<5), rows 3:6 the num quadrant (a>=32, n>=5),
    # rows 6:7 put -1e5 into the two off-quadrants so exp -> exactly 0.
    ub0[0:3, 500:520] = xsl
    ub0[3:6, 532:552] = xsl
    ub0[6, 520:552] = BF16(1.0)
    ub0[7, 500:532] = BF16(1.0)

    spins_f = np.ascontiguousarray(spins, dtype=f32).reshape(B, NA, NCH, CHP)
    noise_f = np.ascontiguousarray(noise, dtype=f32).reshape(B, NA, NCH, CHP)
    g4 = g.reshape(B, NA, NCH, CHP)

    # bv: b-arg duplicated into both (eb, v2) slots of the (k, slot-64)
    # layout; mw: 1.0 | -0.5 g in the same slots.
    bq = b_arg.reshape(B, NA, NCH, CHP).transpose(0, 3, 2, 1)   # [B,100,4,20]
    gq = (-0.5 * g).reshape(B, NA, NCH, CHP).transpose(0, 3, 2, 1)
    bv = np.zeros((B, CHP, NCH, 64), f32)
    mw = np.zeros((B, CHP, NCH, 64), BF16)
    bv[:, :, :, 0:NA] = bq
    bv[:, :, :, 32:32 + NA] = bq
    mw[:, :, :, 0:NA] = BF16(1.0)
    mw[:, :, :, 32:32 + NA] = gq.astype(BF16)
    bv = bv.reshape(B, CHP, 256)
    mw = mw.reshape(B, CHP, 256)

    in_maps = []
    for core in range(NCORES):
        bi, q = divmod(core, Q)
        xq = xs[QA * q:QA * (q + 1)]
        xqh, xql = hilo(xq)
        ub = ub0.copy()
        ub[0, 552:557] = xqh
        ub[1, 552:557] = xql
        ub[2, 552:557] = xqh
        ub[3, 557:562] = xqh
        ub[4, 557:562] = xql
        ub[5, 557:562] = xqh
        ub[6, 552:557] = BF16(-1e5)
        ub[7, 557:562] = BF16(-1e5)
        sgn = np.empty((CHP, 3, NCH, QA), f32)
        sgn[:, 0] = spins_f[bi, QA * q:QA * (q + 1)].transpose(2, 1, 0)
        sgn[:, 1] = g4[bi, QA * q:QA * (q + 1)].transpose(2, 1, 0)
        sgn[:, 2] = noise_f[bi, QA * q:QA * (q + 1)].transpose(2, 1, 0)
        in_maps.append({
            "ub": ub,
            "bv": np.ascontiguousarray(bv[bi]),
            "mw": np.ascontiguousarray(mw[bi]),
            "sgn": sgn.reshape(CHP, 60),
        })
    return in_maps


def kernel(grads, spins, pos, noise, trace=False, **run_kwargs):
    global _NC_CACHE, _NC_SEP, LAST_RESULTS

    axes = _lattice_axes(pos)
    if axes is not None:
        if _NC_SEP is None:
            _NC_SEP = _build_sep()
        in_maps = _host_prep_sep(grads, spins, pos, noise, axes)
        res = bass_utils.run_bass_kernel_spmd(
            _NC_SEP, in_maps, core_ids=list(range(NCORES)), trace=trace,
            **run_kwargs
        )
        LAST_RESULTS = res
        out = np.empty((B, NA, NBC), np.float32)
        for core in range(NCORES):
            bi, q = divmod(core, Q)
            o = np.asarray(res.results[core]["out"], dtype=np.float32)
            out[bi, QA * q:QA * (q + 1)] = (
                o.reshape(CHP, NCH, QA).transpose(2, 1, 0).reshape(QA, NBC))
        return out.reshape(B, L, L, L)

    if _NC_CACHE is None:
        _NC_CACHE = _build_program()
    nc = _NC_CACHE

    in_maps = _host_prep(grads, spins, pos, noise)
    res = bass_utils.run_bass_kernel_spmd(
        nc, in_maps, core_ids=list(range(NCORES)), trace=trace, **run_kwargs
    )
    LAST_RESULTS = res

    out = np.empty((B, N), np.float32)
    for core in range(NCORES):
        bi, q = divmod(core, Q)
        o = np.asarray(res.results[core]["out"], dtype=np.float32).reshape(IPAD)
        out[bi, q * IPC:(q + 1) * IPC] = o[:IPC]
    return out.reshape(B, L, L, L)

